# revision 28
# baseline (speedup 1.0000x reference)
"""Trainium2 Bass kernel for EncryptedFeedForward (poly-activation MLP).

  out = poly(x @ W1 + b1) @ W2 + b2,  poly(z) = 0.5z + 0.5z^2 - 0.125z^3

Sharding: pure data-parallel over the 8192-token axis -> 1024 tokens/core on
8 cores, no collectives. Per core both GEMMs run in transposed orientation
(h.T = W1.T @ x.T, out.T = W2.T @ h.T) so every operand streams with the
contraction dim on SBUF partitions with zero on-chip transposes; the host
pre-tiles inputs / re-assembles the output.

Matmuls run in float32r (fp32 rounded to 11 mantissa bits, full PE rate at
moving-dim 512 vs 4x slower plain fp32). walrus requires fp32r matmul
operands to be produced by a rounding compute op, so weights/x are DMA'd as
raw fp32 bytes into fp32r tiles and converted in place on DVE; h is written
as fp32r directly by the final activation multiply.

poly is folded into PSUM eviction via poly(z) = z * (1 - 0.125*(z-2)^2):
two ACT passes (Square with bias sqrt(.125)*(b1-2), Identity with bias b1)
plus two DVE passes (q = 1 - s, h = z*q).
"""

import numpy as np


def _ensure_concourse():
    """Make the concourse/bass framework importable even if PYTHONPATH is
    not set in the calling environment."""
    try:
        import concourse  # noqa: F401
        return
    except ImportError:
        pass
    import sys
    for p in ("/root/.axon_site", "/root/.axon_site/_ro/trn_rl_repo",
              "/root/.axon_site/_ro/pypackages", "/opt/trn_rl_repo"):
        if p not in sys.path:
            sys.path.append(p)
    import concourse  # noqa: F401


N_CORES = 8
N, D, F = 8192, 1024, 4096
T = N // N_CORES        # tokens per core
P = 128
KO_D = D // P           # 8 k-tiles over D
KO_F = F // P           # 32 k-tiles over F
TCH = 512               # moving-dim chunk (fp32 max 512)
NT = T // TCH           # 2 chunks of tokens

_CACHE = {}


def _build_nc(mm_dtype_name="float32r", loop_n=None, phases="12",
              evict_mode="full"):
    """Build + compile the per-core Bass program. Returns the Bacc object.

    loop_n: if set, wrap the compute body in a hardware For_i loop running
    loop_n times (timing builds only; the production kernel uses None).
    phases/evict_mode: diagnostic knobs ("12"/"1"/"2", "full"/"copy").
    """
    _ensure_concourse()
    import contextlib
    import concourse.mybir as mybir
    import concourse.tile as tile
    from concourse import bacc

    dt = mybir.dt
    AF = mybir.ActivationFunctionType
    mm_dt = getattr(dt, mm_dtype_name)

    nc = bacc.Bacc("TRN2", target_bir_lowering=False, debug=False,
                   num_devices=N_CORES)

    # Host-pre-tiled DRAM layouts (all contiguous per partition). The matmul
    # operands are declared float32r in DRAM; the host pre-rounds them (fp32r
    # == fp32 with the low 12 mantissa bits rounded away, same IEEE layout).
    xT = nc.dram_tensor("xT", [P, KO_D, T], mm_dt, kind="ExternalInput").ap()
    w1 = nc.dram_tensor("w1", [P, KO_F, KO_D, P], mm_dt, kind="ExternalInput").ap()
    b1 = nc.dram_tensor("b1", [P, KO_F], dt.float32, kind="ExternalInput").ap()
    w2 = nc.dram_tensor("w2", [P, KO_F, D], mm_dt, kind="ExternalInput").ap()
    b2 = nc.dram_tensor("b2", [P, KO_D], dt.float32, kind="ExternalInput").ap()
    outT = nc.dram_tensor("outT", [P, KO_D, T], dt.float32, kind="ExternalOutput").ap()

    S = float(np.sqrt(0.125))

    def load_round(pool, dram_ap, shape, tag, name, engine=None):
        tl = pool.tile(shape, mm_dt, tag=tag, name=name)
        (engine or nc.sync).dma_start(tl[:], dram_ap)
        return tl

    with tile.TileContext(nc) as tc:
        with (
            tc.tile_pool(name="const", bufs=1) as const,
            tc.tile_pool(name="xpool", bufs=1) as xpool,
            tc.tile_pool(name="hpool", bufs=1) as hpool,
            tc.tile_pool(name="w1pool", bufs=2) as w1pool,
            tc.tile_pool(name="w2pool", bufs=2) as w2pool,
            tc.tile_pool(name="evict", bufs=2) as evict,
            tc.tile_pool(name="opool", bufs=2) as opool,
            tc.tile_pool(name="psum", bufs=8, space="PSUM") as psum,
        ):
            # --- constants ---
            b1_sb = const.tile([P, KO_F], dt.float32, tag="b1")
            nc.sync.dma_start(b1_sb[:], b1[:])
            b2_sb = const.tile([P, KO_D], dt.float32, tag="b2")
            nc.sync.dma_start(b2_sb[:], b2[:])
            # bias for the Square pass: sqrt(.125) * (b1 - 2)
            b1s_sb = const.tile([P, KO_F], dt.float32, tag="b1s")
            nc.vector.tensor_scalar(b1s_sb[:], b1_sb[:], S, -2.0 * S,
                                    mybir.AluOpType.mult, mybir.AluOpType.add)

            # --- x.T resident in SBUF (host pre-rounded fp32r) ---
            # chunked by k-tile so the first matmul only waits for x[k=0]
            x_mm = xpool.tile([P, KO_D, T], mm_dt, tag="xT", name="x_sb")
            for k in range(KO_D):
                nc.sync.dma_start(x_mm[:, k], xT[:, k])

            # --- h.T resident buffer [P, KO_F, T], produced as fp32r ---
            h_mm = hpool.tile([P, KO_F, T], mm_dt, tag="hT", name="h_sb")

            loop_cm = (tc.For_i(0, loop_n, 1,
                                hint_engines=(mybir.EngineType.PE,
                                              mybir.EngineType.Activation,
                                              mybir.EngineType.DVE,
                                              mybir.EngineType.SP))
                       if loop_n is not None else contextlib.nullcontext())
            with loop_cm:
                _emit_body(nc, tc, mybir, mm_dt,
                           w1pool, w2pool, evict, opool, psum,
                           b1_sb, b1s_sb, b2_sb, x_mm, h_mm,
                           w1, w2, outT, load_round,
                           phases=phases, evict_mode=evict_mode)

    nc.compile()
    return nc


def _emit_body(nc, tc, mybir, mm_dt, w1pool, w2pool, evict, opool, psum,
               b1_sb, b1s_sb, b2_sb, x_mm, h_mm, w1, w2, outT, load_round,
               phases="12", evict_mode="full"):
    import numpy as _np
    dt = mybir.dt
    AF = mybir.ActivationFunctionType
    S = float(_np.sqrt(0.125))
    if True:
        if "1" in phases:
            # ---------------- Phase 1: h.T = poly(W1.T @ x.T + b1) ----------
            # w1 loaded 2 f-tiles per DMA (1 MB transfers)
            for fb in range(KO_F // 2):
                w1_mm = load_round(w1pool, w1[:, 2 * fb:2 * fb + 2],
                                   [P, 2, KO_D, P], "w1t", f"w1_{fb}")
                for fi in range(2):
                    ft = 2 * fb + fi
                    # both t-chunks accumulate in parallel banks so the two
                    # matmuls per k share the stationary operand back-to-back
                    pst = [psum.tile([P, TCH], dt.float32, tag="ps",
                                     name=f"ps1_{ft}_{t}") for t in range(NT)]
                    for k in range(KO_D):
                        for t in range(NT):
                            ts = slice(t * TCH, (t + 1) * TCH)
                            nc.tensor.matmul(pst[t][:], w1_mm[:, fi, k, :],
                                             x_mm[:, k, ts],
                                             start=(k == 0), stop=(k == KO_D - 1))
                    for t in range(NT):
                        ts = slice(t * TCH, (t + 1) * TCH)
                        ps = pst[t]
                        if evict_mode == "copy":
                            nc.scalar.activation(h_mm[:, ft, ts], ps[:],
                                                 AF.Identity,
                                                 bias=b1_sb[:, ft:ft + 1],
                                                 scale=1.0)
                            continue
                        # engine-balanced eviction: ACT does the Square,
                        # DVE does z and the final multiply, GpSimd does q.
                        # s = 0.125*(z-2)^2 computed straight from PSUM
                        s_sb = evict.tile([P, TCH], dt.float32, tag="s")
                        nc.scalar.activation(s_sb[:], ps[:], AF.Square,
                                             bias=b1s_sb[:, ft:ft + 1], scale=S)
                        # z = psum + b1
                        z_sb = evict.tile([P, TCH], dt.float32, tag="z")
                        nc.vector.tensor_scalar_add(z_sb[:], ps[:],
                                                    b1_sb[:, ft:ft + 1])
                        # q = 1 - s  (in place)
                        nc.vector.tensor_scalar(s_sb[:], s_sb[:], -1.0, 1.0,
                                                mybir.AluOpType.mult,
                                                mybir.AluOpType.add)
                        # h = z * q -> resident h.T (fp32r rounded on write)
                        nc.vector.tensor_mul(h_mm[:, ft, ts], z_sb[:], s_sb[:])

            # ---------------- Phase 2: out.T = W2.T @ h.T + b2 --------------
            # d-split groups: each group's (2 t-chunks x 1 d-tile) lives in
            # 2 PSUM banks, so w2 streams exactly once AND up to 4 groups
            # overlap (evictions of one against matmuls of the next).
            DPQ = KO_D // 8      # 1 d-tile per group
            KB2 = 16             # k-tiles per w2 load (1 MB DMA)
            for dq in range(8) if "2" in phases else []:
                ds0 = dq * DPQ * P
                pss = [psum.tile([P, TCH], dt.float32, tag="ps",
                                 name=f"ps2_{dq}_{t}_{d}")
                       for t in range(NT) for d in range(DPQ)]
                for kb in range(KO_F // KB2):
                    w2_mm = load_round(
                        w2pool, w2[:, kb * KB2:(kb + 1) * KB2,
                                   ds0:ds0 + DPQ * P],
                        [P, KB2, DPQ * P], "w2t", f"w2_{dq}_{kb}",
                        engine=nc.scalar)
                    for kk in range(KB2):
                        k = kb * KB2 + kk
                        for d in range(DPQ):
                            for t in range(NT):
                                ts = slice(t * TCH, (t + 1) * TCH)
                                nc.tensor.matmul(
                                    pss[t * DPQ + d][:],
                                    w2_mm[:, kk, d * P:(d + 1) * P],
                                    h_mm[:, k, ts],
                                    start=(k == 0), stop=(k == KO_F - 1))
                for t in range(NT):
                    ts = slice(t * TCH, (t + 1) * TCH)
                    for d in range(DPQ):
                        o_sb = opool.tile([P, TCH], dt.float32, tag="o")
                        nc.scalar.activation(o_sb[:], pss[t * DPQ + d][:],
                                             AF.Identity,
                                             bias=b2_sb[:, dq * DPQ + d:
                                                        dq * DPQ + d + 1],
                                             scale=1.0)
                        nc.sync.dma_start(outT[:, dq * DPQ + d, ts], o_sb[:])


def get_nc(mm_dtype_name="float32r", loop_n=None, phases="12",
           evict_mode="full"):
    key = (mm_dtype_name, loop_n, phases, evict_mode)
    if key not in _CACHE:
        _CACHE[key] = _build_nc(mm_dtype_name, loop_n=loop_n, phases=phases,
                                evict_mode=evict_mode)
    return _CACHE[key]


def _round_fp32r(a):
    """Round fp32 to the fp32r grid: round-to-nearest-even at 11 mantissa
    bits, low 12 bits zeroed (same IEEE fp32 bit layout)."""
    a = np.ascontiguousarray(a, dtype=np.float32)
    u = a.view(np.uint32)
    lsb = (u >> 12) & 1
    r = ((u + 0x7FF + lsb) & 0xFFFFF000).astype(np.uint32)
    return r.view(np.float32)


def _prep_shared(W1, b1, W2, b2):
    """Host-side re-tiling of the weights (shared by all cores)."""
    W1 = np.asarray(W1, dtype=np.float32)
    W2 = np.asarray(W2, dtype=np.float32)
    b1 = np.asarray(b1, dtype=np.float32)
    b2 = np.asarray(b2, dtype=np.float32)
    # w1 [D,F] -> [P, KO_F, KO_D, P]: (ko p) (ft fi) -> p ft ko fi
    w1_dev = _round_fp32r(
        W1.reshape(KO_D, P, KO_F, P).transpose(1, 2, 0, 3))
    # w2 [F,D] -> [P, KO_F, D]: (ko p) d -> p ko d
    w2_dev = _round_fp32r(
        W2.reshape(KO_F, P, D).transpose(1, 0, 2))
    b1_dev = np.ascontiguousarray(b1.reshape(KO_F, P).T)
    b2_dev = np.ascontiguousarray(b2.reshape(KO_D, P).T)
    return w1_dev, w2_dev, b1_dev, b2_dev


def _prep_x_shard(x_c):
    # x shard [T, D] -> x.T tiled [P, KO_D, T]: (t) (ko p) -> p ko t
    return _round_fp32r(x_c.T.reshape(KO_D, P, T).transpose(1, 0, 2))


def _unprep_out(outT_dev):
    # [P, KO_D, T] -> out shard [T, D]
    return np.ascontiguousarray(
        outT_dev.transpose(1, 0, 2).reshape(D, T).T)


def kernel(x, W1, b1, W2, b2):
    _ensure_concourse()
    from concourse.bass_utils import run_bass_kernel_spmd

    x = np.asarray(x, dtype=np.float32)
    nc = get_nc()
    w1_dev, w2_dev, b1_dev, b2_dev = _prep_shared(W1, b1, W2, b2)

    in_maps = []
    for c in range(N_CORES):
        in_maps.append({
            "xT": _prep_x_shard(x[c * T:(c + 1) * T]),
            "w1": w1_dev,
            "b1": b1_dev,
            "w2": w2_dev,
            "b2": b2_dev,
        })
    res = run_bass_kernel_spmd(nc, in_maps, core_ids=list(range(N_CORES)))
    out = np.empty((N, D), dtype=np.float32)
    for c in range(N_CORES):
        out[c * T:(c + 1) * T] = _unprep_out(res.results[c]["outT"])
    return out
